# revision 30
# baseline (speedup 1.0000x reference)
"""Trainium2 Bass kernel: DynamicMoERoutingLayer (moe_routing).

Reference computes: routing projection -> cosine-sim vs 10 expert embeddings ->
softmax weights -> 10 expert 3x3 VALID convs -> weighted combine.

Key algebraic rewrite: conv is linear in its weights, so
    sum_n w[b,n] * conv(x_b, W_n)  ==  conv(x_b, sum_n w[b,n] * W_n)
We therefore combine the 10 expert kernels into ONE per-image kernel on device
(10x less conv compute), then run a single 3x3 conv per image.

Distribution: data-parallel over batch, 4 images per core (8 cores).

Conv-as-matmul with row-pair K-packing: each image's x lives in a [128, 4160]
bf16 tile X2 whose partitions 0-63 hold the 64 input channels (flat pixel
space y*64+x) and partitions 64-127 hold the SAME channels shifted one image
row (+64 px).  A matmul at column offset dx then contracts kernel rows 0 AND 1
in one K=128 pass; kernel row 2 needs a separate K=64 pass at offset 128+dx.
So a 3x3 conv costs 6 matmul slots per 512-px chunk instead of 9.
Two images (A, B) run concurrently on disjoint PE-array column halves
(tile_position (0,0) vs (0,64)), so the array stays fully busy.

v2 front-latency restructure (the v1 kernel spent 39us of 68us before the
first conv matmul):
- X2 tiles are packed on the HOST (pre-cast bf16, pre-shifted, pre-padded)
  and DMA'd directly: no on-device fp32->bf16 casts (was 15us of ScalarE),
  no SBUF->SBUF shift DMAs, and half the HBM bytes for x.
- base (expert conv weights in stacked-tap layout) uploads in bf16 (1MB not
  2MB); the combined-weight MACs accumulate in fp32 on VectorE (rows 0-1
  taps) and GpSimdE (row-2 taps) in parallel, reading the routing weights
  straight from PSUM (no broadcast-copy hop).
- The routing chain drops the r/emb transposes (||r|| comes from the diagonal
  of a PE Gram matmul), uses ALU divide instead of reciprocal+mult hops, and
  skips the softmax max-subtraction (cosine sims are bounded by 1).
- The constant blob is split so the 273KB the routing projection needs lands
  first; x and base follow in compute order.
- Output drains PSUM->SBUF as bf16 (host upcasts) halving the store traffic;
  the last chunk is 384 px wide (flat rows 62-63 are VALID-conv garbage).

The flat-pixel formulation computes 64x62 output positions per image; the
garbage columns (VALID conv is 62x62) are trimmed on the host.

Hardware/toolchain constraints honored:
- A Matmult can carry only ONE semaphore wait (walrus ISA): Bacc's
  compile() passes legalize the rest, and PE-queue NOPs with dependency APs
  (the Tile-sanctioned mechanism) absorb cross-engine waits up front.
- HWDGE DMA descriptors only get the fast 16-engine spray for full-tile-width
  destinations with non-overlapping source rows; everything else runs ~10x
  slower, so all loads are shaped accordingly.
- fp32r matmuls are rejected at tile_position != (0,0), hence bf16.
"""

import functools
import os
import sys

import numpy as np

for _p in ("/opt/trn_rl_repo",):
    if os.path.isdir(_p) and _p not in sys.path:
        sys.path.insert(0, _p)

import ml_dtypes

import concourse.bacc as bacc
import concourse.bass as bass
import concourse.mybir as mybir
import concourse.tile as tile
from concourse.bass_utils import run_bass_kernel_spmd

FP = mybir.dt.float32
BF = mybir.dt.bfloat16
AF = mybir.ActivationFunctionType
OP = mybir.AluOpType
BF_NP = ml_dtypes.bfloat16

N_CORES = 8
B = 32
B_LOC = B // N_CORES          # images per core
NPAIR = B_LOC // 2
CIN = 64
COUT = 64
PIX = 64 * 64                 # flat pixels computed per image (incl. garbage)
XCOLS = 4160                  # X2 columns (max read col 4097, zero-padded)
NEXP = 10
D = 128
R = 512
CWF = 384                     # combined weights: 192 pair-taps + 192 row-2
CW_V = 192                    # VectorE's MAC share: rows-0/1 taps, 128 parts
# chunk list: (col offset, width). Last chunk is 384 wide: flat rows 62/63
# (cols 3968:4096) are VALID-conv garbage, never stored.
CHUNKS = [(c * 512, 512) for c in range(7)] + [(3584, 384)]
WAVE = 3                      # chunks per wave (PSUM banks: 6 conv + 2 routing)

# csta columns (fp32): what the routing projection needs, lands first
CA_RPW = 0                    # [128, 4, 128]
CA_RV = 512                   # [128, 4, 4]
CA_RPB = 528                  # [128, 1]
CA_DIAG = 529                 # [4, 4] identity mask (Gram diagonal extract)
CA_COLS = 533
# cstb columns (fp32): the rest of the routing constants
CB_ID = 0                     # [16, 16] identity (10x10 used for transpose)
CB_EMB = 16                   # [10, 128] on partitions 0..9
CB_SELP = 144                 # [4, 2, 128] pair selector, partitions 0..3
CB_SELI = 400                 # [4, 4, 128] image selector, partitions 0..3
CB_CBT = 912                  # [128, 10] conv_b.T tiled 2x
CB_COLS = 922


def build_nc():
    # Bacc (not raw Bass): its compile() runs move_matmul_waits_to_ldweights +
    # generate_event_semaphores, which legalize multi-wait instructions for
    # the walrus ISA (each instruction carries at most one sync wait).
    nc = bacc.Bacc(None)

    x2_d = nc.dram_tensor("x2", [B_LOC * 128, XCOLS], BF, kind="ExternalInput")
    csta_d = nc.dram_tensor("csta", [128, CA_COLS], FP, kind="ExternalInput")
    cstb_d = nc.dram_tensor("cstb", [128, CB_COLS], FP, kind="ExternalInput")
    identb_d = nc.dram_tensor("identb", [128, 128], BF, kind="ExternalInput")
    base_d = nc.dram_tensor("base", [128, NEXP, CWF], BF,
                            kind="ExternalInput")
    out_d = nc.dram_tensor("out", [B_LOC, COUT, PIX], BF, kind="ExternalOutput")

    with tile.TileContext(nc) as tc:
        with (
            tc.tile_pool(name="consts", bufs=1) as consts,
            tc.tile_pool(name="x2p", bufs=4) as x2p,
            tc.tile_pool(name="cwbp", bufs=4) as cwbp,
            tc.tile_pool(name="outp", bufs=2) as outp,
            tc.tile_pool(name="scr", bufs=1) as scr,
            tc.tile_pool(name="rps", bufs=2, space="PSUM") as rps,
            tc.tile_pool(name="cwps", bufs=1, space="PSUM") as cwps,
            tc.tile_pool(name="cps", bufs=5, space="PSUM") as cps,
        ):
            # activation-table warmup: pulls the lazy Sqrt/Exp table loads
            # (1.3 us each) off the routing critical path.  Exp first: the
            # routing path uses Sqrt (twice) before its Exp, so warm in the
            # order that leaves Sqrt resident.  memset on GpSimd: it is idle
            # at startup, while VectorE's queue would delay the warmup.
            warm = scr.tile([1, 1], FP)
            nc.gpsimd.memset(warm, 1.0)
            nc.scalar.activation(out=warm, in_=warm, func=AF.Exp)
            nc.scalar.activation(out=warm, in_=warm, func=AF.Sqrt)

            # ---- input DMAs ---------------------------------------------
            # csta (the routing projection's inputs) goes ALONE first: the
            # DMA engines share ~280GB/s across all in-flight transfers, and
            # letting x2/base contend pushed csta (and thus the whole
            # routing->weights->conv chain) 4us later.  A sync-queue NOP
            # gates the bulk issues on csta's completion.
            csta = consts.tile([128, CA_COLS], FP)
            nc.sync.dma_start(out=csta, in_=csta_d[:])
            cstb = consts.tile([128, CB_COLS], FP)
            identb = consts.tile([128, 128], BF)
            base_t = consts.tile([128, NEXP, CWF], BF)
            x2 = [x2p.tile([128, XCOLS], BF, name="x2", tag="x2")
                  for _ in range(B_LOC)]
            # the gate NOP "writes" one element of every gated destination:
            # writer-writer ordering is what actually binds under the
            # work-conserving scheduler (a bare NOP just gets reordered).
            dgate = mybir.InstNoOp(
                name=nc.get_next_instruction_name(), text_hint="dgate",
                ins=[nc.sync.lower_ap(csta[:, 0:1])],
                outs=[nc.sync.lower_ap(t[0:1, 0:1])
                      for t in (cstb, identb, base_t, *x2)])
            nc.sync.add_instruction(dgate)
            nc.sync.dma_start(out=cstb, in_=cstb_d[:])
            nc.sync.dma_start(out=identb, in_=identb_d[:])
            nc.sync.dma_start(out=base_t, in_=base_d[:])
            xfull = x2_d[:]
            for i in range(B_LOC):
                nc.sync.dma_start(out=x2[i], in_=bass.AP(
                    tensor=xfull.tensor,
                    offset=xfull.offset + i * 128 * XCOLS,
                    ap=[[XCOLS, 128], [1, XCOLS]]))

            rpw_t = csta[:, CA_RPW:CA_RPW + 512].rearrange(
                "p (k d) -> p k d", k=4)
            rv_t = csta[:, CA_RV:CA_RV + 16].rearrange("p (k b) -> p k b", k=4)
            rpb_t = csta[:, CA_RPB:CA_RPB + 1]
            diag_t = csta[0:B_LOC, CA_DIAG:CA_DIAG + B_LOC]
            ident = cstb[0:16, CB_ID:CB_ID + 16]
            emb_t = cstb[0:NEXP, CB_EMB:CB_EMB + 128]
            selp_t = cstb[0:B_LOC, CB_SELP:CB_SELP + 256].rearrange(
                "b (p q) -> b p q", p=NPAIR)
            seli_t = cstb[0:B_LOC, CB_SELI:CB_SELI + 512].rearrange(
                "b (i q) -> b i q", i=B_LOC)
            cbt_t = cstb[:, CB_CBT:CB_CBT + NEXP]

            # normalized embeddings FIRST on the Vector queue (they only
            # need cstb, which lands ~2us before the rpw matmuls finish --
            # emitting them after radd cost 3.5us of head-of-line blocking)
            esq = scr.tile([NEXP, D], FP)
            en2 = scr.tile([NEXP, 1], FP)
            nc.vector.scalar_tensor_tensor(out=esq, in0=emb_t, scalar=1.0,
                                           in1=emb_t, op0=OP.mult, op1=OP.mult,
                                           accum_out=en2)
            enorm = scr.tile([NEXP, 1], FP)
            nc.scalar.activation(out=enorm, in_=en2, func=AF.Sqrt)
            einv = scr.tile([NEXP, 1], FP)
            nc.vector.reciprocal(einv, enorm)
            ehat = scr.tile([NEXP, D], FP)
            nc.vector.tensor_scalar(out=ehat, in0=emb_t, scalar1=einv,
                                    scalar2=None, op0=OP.mult)

            # ---- routing: r = rv @ rp_w.T + rp_b  (D on partitions) -------
            r_ps = rps.tile([128, B_LOC], FP, tag="r")
            for k0 in range(R // 128):
                nc.tensor.matmul(r_ps, lhsT=rpw_t[:, k0, :], rhs=rv_t[:, k0, :],
                                 start=(k0 == 0), stop=(k0 == R // 128 - 1))
            # emb transpose next on PE: its input is ready before radd lands
            ehatT_ps = rps.tile([D, NEXP], FP, tag="r")
            nc.tensor.transpose(ehatT_ps, ehat, ident[:NEXP, :NEXP])
            ehatT = scr.tile([D, NEXP], FP)
            nc.vector.tensor_copy(ehatT, ehatT_ps)
            rT = scr.tile([128, B_LOC], FP)
            nc.vector.tensor_scalar(out=rT, in0=r_ps, scalar1=rpb_t,
                                    scalar2=None, op0=OP.add)

            # ||r_b||^2 from the diagonal of the Gram matrix rT.T @ rT
            # (avoids the PE transpose + PSUM copy + square of v1)
            gram_ps = rps.tile([B_LOC, B_LOC], FP, tag="r")
            nc.tensor.matmul(gram_ps, lhsT=rT, rhs=rT, start=True, stop=True)
            gscrap = scr.tile([B_LOC, B_LOC], FP)
            rn2 = scr.tile([B_LOC, 1], FP)
            nc.vector.scalar_tensor_tensor(out=gscrap, in0=gram_ps, scalar=1.0,
                                           in1=diag_t, op0=OP.mult,
                                           op1=OP.mult, accum_out=rn2)
            rnorm = scr.tile([B_LOC, 1], FP)
            nc.scalar.activation(out=rnorm, in_=rn2, func=AF.Sqrt)

            # cosine sim [b, n]: the 1/||r|| scale is FUSED into the Exp
            # activation (per-partition scale, reading the dot straight from
            # PSUM); softmax skips the max-subtraction (|sim| <= 1).
            dot_ps = rps.tile([B_LOC, NEXP], FP, tag="r")
            nc.tensor.matmul(dot_ps, lhsT=rT, rhs=ehatT, start=True, stop=True)
            rinv = scr.tile([B_LOC, 1], FP)
            nc.vector.reciprocal(rinv, rnorm)
            ex = scr.tile([B_LOC, NEXP], FP)
            sume = scr.tile([B_LOC, 1], FP)
            nc.scalar.activation(out=ex, in_=dot_ps, func=AF.Exp,
                                 scale=rinv[:, 0:1], accum_out=sume)
            # softmax normalization is FOLDED into the output drain (psum *
            # sinv_pair + bias) and the bias accumulate: the selector
            # broadcasts consume the raw exp() weights.

            # routing weights broadcast to all 128 partitions via selector
            # matmuls, in consumption order.  TWO psum tiles (img0, img1 |
            # img2, img3, pair0, pair1) so the first pair's combines see
            # their dependency satisfied after two matmuls, not six
            # (sub-tile writer tracking is tile-granular).
            w128a_ps = rps.tile([128, 2, NEXP], FP, tag="r")
            w128b_ps = rps.tile([128, 4, NEXP], FP, tag="r")
            nc.tensor.matmul(w128a_ps[:, 0, :], lhsT=seli_t[:, 0, :],
                             rhs=ex, start=True, stop=True)
            nc.tensor.matmul(w128a_ps[:, 1, :], lhsT=seli_t[:, 1, :],
                             rhs=ex, start=True, stop=True)
            nc.tensor.matmul(w128b_ps[:, 0, :], lhsT=seli_t[:, 2, :],
                             rhs=ex, start=True, stop=True)
            nc.tensor.matmul(w128b_ps[:, 1, :], lhsT=seli_t[:, 3, :],
                             rhs=ex, start=True, stop=True)
            nc.tensor.matmul(w128b_ps[:, 2, :], lhsT=selp_t[:, 0, :],
                             rhs=ex, start=True, stop=True)
            nc.tensor.matmul(w128b_ps[:, 3, :], lhsT=selp_t[:, 1, :],
                             rhs=ex, start=True, stop=True)

            # img0's raw weights staged to SBUF for the ScalarE wI builds --
            # FIRST on the Vector queue: it gates the PE-side img0 combine.
            sinvp = consts.tile([128, NPAIR], FP)
            sump = scr.tile([128, NPAIR], FP)
            w128sb = consts.tile([128, NEXP], FP)
            nc.vector.tensor_copy(w128sb, w128a_ps[:, 0, :])

            # ---- combined conv weights ----------------------------------
            # img0 on the (otherwise idle) PE: 10 PSUM-accumulating matmuls
            # with lhsT = w_n * I (built by ScalarE from the bf16 identity),
            # rhs = base_n.  Runs concurrently with img1's VectorE MAC chain,
            # halving the latency to the first conv matmul.
            wip = scr.tile([128, NEXP, 128], BF)
            cw_ps = cwps.tile([128, CWF], FP, name="cw0")
            for n in range(NEXP):
                nc.scalar.activation(out=wip[:, n, :], in_=identb,
                                     func=AF.Copy, scale=w128sb[:, n:n + 1])
                nc.tensor.matmul(cw_ps, lhsT=wip[:, n, :],
                                 rhs=base_t[:, n, :],
                                 start=(n == 0), stop=(n == NEXP - 1))
            cwb0 = cwbp.tile([128, CWF], BF, name="cwb", tag="cwb")
            nc.scalar.activation(out=cwb0, in_=cw_ps, func=AF.Copy)

            # imgs 1-3 as all-bf16 MAC chains on VectorE, reading the raw
            # weights straight from PSUM.  NOP gates keep the work-conserving
            # scheduler from interleaving the chains (which would delay
            # img1, the conv gate, by several us).  The pair-stacked softmax
            # denominators (free-axis sum of the pair selector outputs) and
            # the folded-normalization bias accumulates slot in after the
            # img1/img2 chains -- their consumers (the drains) run much
            # later.
            bias2 = consts.tile([128, NPAIR], FP)
            bscrap = scr.tile([128, NEXP], FP)
            bscrap2 = scr.tile([128, NEXP], FP)
            wslot = [(w128a_ps, 1), (w128b_ps, 0), (w128b_ps, 1)]
            cwb = [cwb0]
            for i in (1, 2, 3):
                wtile, slot = wslot[i - 1]
                cwbi = cwbp.tile([128, CWF], BF, name="cwb", tag="cwb")
                if i >= 2:
                    # serialize img2/img3's chains behind img1's (the conv
                    # gate): a 1-element copy from cwb1 into this tile is a
                    # writer-writer ordering the scheduler cannot hoist --
                    # a bare NOP gate gets reordered and the interleaved
                    # chains delayed cwb1 by 3.5us.  img2 and img3 still
                    # interleave with each other, which pipelines their
                    # semaphore latency.
                    nc.vector.tensor_copy(cwbi[0:1, 0:1], cwb[1][0:1, 0:1])
                nc.vector.tensor_scalar(out=cwbi, in0=base_t[:, 0, :],
                                        scalar1=wtile[:, slot, 0:1],
                                        scalar2=None, op0=OP.mult)
                for n in range(1, NEXP):
                    nc.vector.scalar_tensor_tensor(
                        out=cwbi, in0=base_t[:, n, :],
                        scalar=wtile[:, slot, n:n + 1],
                        in1=cwbi, op0=OP.mult, op1=OP.add)
                cwb.append(cwbi)
                if i == 1:
                    nc.vector.tensor_reduce(
                        out=sump[:, 0:1], in_=w128b_ps[:, 2, :],
                        axis=mybir.AxisListType.X, op=OP.add)
                    nc.vector.reciprocal(sinvp[:, 0:1], sump[:, 0:1])
                    nc.vector.scalar_tensor_tensor(
                        out=bscrap, in0=w128b_ps[:, 2, :],
                        scalar=sinvp[:, 0:1],
                        in1=cbt_t, op0=OP.mult, op1=OP.mult,
                        accum_out=bias2[:, 0:1])
                if i == 2:
                    nc.vector.tensor_reduce(
                        out=sump[:, 1:2], in_=w128b_ps[:, 3, :],
                        axis=mybir.AxisListType.X, op=OP.add)
                    nc.vector.reciprocal(sinvp[:, 1:2], sump[:, 1:2])
                    nc.vector.scalar_tensor_tensor(
                        out=bscrap2, in0=w128b_ps[:, 3, :],
                        scalar=sinvp[:, 1:2],
                        in1=cbt_t, op0=OP.mult, op1=OP.mult,
                        accum_out=bias2[:, 1:2])

            # ---- per-pair conv ------------------------------------------
            for p in range(NPAIR):
                iA, iB = 2 * p, 2 * p + 1
                outt = outp.tile([128, PIX], BF)
                dst = out_d[2 * p:2 * p + 2].flatten_outer_dims()
                for w0 in range(0, len(CHUNKS), WAVE):
                    chunks = CHUNKS[w0:w0 + WAVE]
                    pst = {c: cps.tile([128, w], FP, name="pst")
                           for (c, w) in chunks}
                    # PE-queue NOP absorbs all cross-engine waits (psum bank
                    # release, X2 DMAs, cwb MACs) so each Matmult needs at
                    # most its single legal wait.  x2/cwb edges only on the
                    # first wave: later waves are ordered behind it on the
                    # in-order PE queue, and fewer event-semaphore edges
                    # shorten the fixed end-of-NEFF semaphore-reset teardown.
                    ins = []
                    if w0 == 0:
                        ins = [nc.tensor.lower_ap(x2[iA][:, 0:1]),
                               nc.tensor.lower_ap(x2[iB][:, 0:1]),
                               nc.tensor.lower_ap(cwb[iA][:, 0:1]),
                               nc.tensor.lower_ap(cwb[iB][:, 0:1])]
                    dep = mybir.InstNoOp(
                        name=nc.get_next_instruction_name(), text_hint="dep",
                        ins=ins,
                        outs=[nc.tensor.lower_ap(pst[c]) for (c, w) in chunks],
                    )
                    nc.tensor.add_instruction(dep)
                    # phase 1: kernel rows 0+1 in one K=128 pass per dx
                    for dx in range(3):
                        for (c, w) in chunks:
                            lo = c + dx
                            for half, img in ((0, iA), (1, iB)):
                                sl = slice(64 * half, 64 * half + 64)
                                nc.tensor.matmul(
                                    pst[c][sl, :],
                                    lhsT=cwb[img][0:128, dx * 64:dx * 64 + 64],
                                    rhs=x2[img][0:128, lo:lo + w],
                                    start=(dx == 0), stop=False,
                                    skip_group_check=True)
                    # phase 2: kernel row 2, K=64 from the top half only
                    # (weights always on array rows 0-63: tile positions
                    # beyond (0,0)/(0,64) proved unreliable on silicon)
                    for dx in range(3):
                        for (c, w) in chunks:
                            lo = c + 128 + dx
                            for half, img in ((0, iA), (1, iB)):
                                sl = slice(64 * half, 64 * half + 64)
                                nc.tensor.matmul(
                                    pst[c][sl, :],
                                    lhsT=cwb[img][0:64,
                                                  192 + dx * 64:256 + dx * 64],
                                    rhs=x2[img][0:64, lo:lo + w],
                                    start=False, stop=(dx == 2),
                                    skip_group_check=True)
                    for (c, w) in chunks:
                        nc.scalar.activation(
                            out=outt[:, c:c + w],
                            in_=pst[c], func=AF.Identity,
                            bias=bias2[:, p:p + 1],
                            scale=sinvp[:, p:p + 1])
                    lo = chunks[0][0]
                    hi = chunks[-1][0] + chunks[-1][1]
                    nc.sync.dma_start(out=dst[:, lo:hi], in_=outt[:, lo:hi])

    nc.compile()
    return nc


@functools.lru_cache(maxsize=1)
def _nc_cached():
    return build_nc()


def _prep_in_maps(inputs):
    x = np.asarray(inputs["x"], dtype=np.float32).reshape(B, CIN, PIX)
    rv = np.asarray(inputs["routing_vector"], dtype=np.float32)
    conv_w = np.asarray(inputs["conv_w"], dtype=np.float32)
    conv_b = np.asarray(inputs["conv_b"], dtype=np.float32)
    emb = np.asarray(inputs["emb"], dtype=np.float32)
    rp_w = np.asarray(inputs["rp_w"], dtype=np.float32)
    rp_b = np.asarray(inputs["rp_b"], dtype=np.float32)

    # base layout for the stacked-tap lhsT (see module docstring):
    #   cols 0:192  : [p = cin + 64*dy(0/1), n, dx*64 + cout]
    #   cols 192:288: [p = cin (0..63),      n, dx*64 + cout]  (kernel row 2)
    base = np.zeros((128, NEXP, CWF), np.float32)
    b01 = conv_w[:, :, :, 0:2, :].transpose(3, 2, 0, 4, 1)  # dy,c,n,dx,m
    base[:, :, 0:192] = b01.reshape(128, NEXP, 192)
    b2 = conv_w[:, :, :, 2, :].transpose(2, 0, 3, 1)        # c,n,dx,m
    base[0:64, :, 192:384] = b2.reshape(64, NEXP, 192)
    base = base.astype(BF_NP)

    csta = np.zeros((128, CA_COLS), np.float32)
    csta[:, CA_RPW:CA_RPW + 512] = (
        rp_w.T.reshape(4, 128, D).transpose(1, 0, 2).reshape(128, 512))
    csta[:, CA_RPB] = rp_b
    csta[0:B_LOC, CA_DIAG:CA_DIAG + B_LOC] = np.eye(B_LOC, dtype=np.float32)

    cstb = np.zeros((128, CB_COLS), np.float32)
    cstb[0:16, CB_ID:CB_ID + 16] = np.eye(16, dtype=np.float32)
    cstb[0:NEXP, CB_EMB:CB_EMB + 128] = emb
    selp = np.zeros((B_LOC, NPAIR, 128), np.float32)
    for p in range(NPAIR):
        selp[2 * p, p, 0:64] = 1.0
        selp[2 * p + 1, p, 64:128] = 1.0
    cstb[0:B_LOC, CB_SELP:CB_SELP + 256] = selp.reshape(B_LOC, 256)
    seli = np.zeros((B_LOC, B_LOC, 128), np.float32)
    for i in range(B_LOC):
        seli[i, i, :] = 1.0
    cstb[0:B_LOC, CB_SELI:CB_SELI + 512] = seli.reshape(B_LOC, 512)
    cstb[:, CB_CBT:CB_CBT + NEXP] = np.tile(conv_b.T, (2, 1))

    in_maps = []
    for c in range(N_CORES):
        sl = slice(B_LOC * c, B_LOC * (c + 1))
        xr = x[sl]                                     # [B_LOC, 64, PIX]
        x2 = np.zeros((B_LOC, 128, XCOLS), np.float32)
        x2[:, 0:64, 0:PIX] = xr
        x2[:, 64:128, 0:PIX - 64] = xr[:, :, 64:]
        ccsta = csta.copy()
        ccsta[:, CA_RV:CA_RV + 16] = (
            rv[sl].T.reshape(4, 128, B_LOC).transpose(1, 0, 2).reshape(128, 16))
        in_maps.append({
            "x2": x2.astype(BF_NP).reshape(B_LOC * 128, XCOLS),
            "csta": ccsta,
            "cstb": cstb,
            "identb": np.eye(128, dtype=BF_NP),
            "base": base,
        })
    return in_maps


def run(inputs, trace=False, **kw):
    """Returns (full_output, BassKernelResults)."""
    nc = _nc_cached()
    in_maps = _prep_in_maps(inputs)
    res = run_bass_kernel_spmd(nc, in_maps, core_ids=list(range(N_CORES)),
                               trace=trace, **kw)
    outs = [np.asarray(r["out"]).astype(np.float32)
            .reshape(B_LOC, COUT, 64, 64)[:, :, :62, :62]
            for r in res.results]
    return np.concatenate(outs, axis=0), res


def kernel(**inputs):
    out, _ = run(inputs, trace=False)
    return out


# revision 33
# speedup vs baseline: 1.0389x; 1.0389x over previous
"""Trainium2 Bass kernel: DynamicMoERoutingLayer (moe_routing).

Reference computes: routing projection -> cosine-sim vs 10 expert embeddings ->
softmax weights -> 10 expert 3x3 VALID convs -> weighted combine.

Key algebraic rewrite: conv is linear in its weights, so
    sum_n w[b,n] * conv(x_b, W_n)  ==  conv(x_b, sum_n w[b,n] * W_n)
We therefore combine the 10 expert kernels into ONE per-image kernel on device
(10x less conv compute), then run a single 3x3 conv per image.

Distribution: data-parallel over batch, 4 images per core (8 cores).

Conv-as-matmul with row-pair K-packing: each image's x lives in a [128, 4160]
bf16 tile X2 whose partitions 0-63 hold the 64 input channels (flat pixel
space y*64+x) and partitions 64-127 hold the SAME channels shifted one image
row (+64 px).  A matmul at column offset dx then contracts kernel rows 0 AND 1
in one K=128 pass; kernel row 2 needs a separate K=64 pass at offset 128+dx.
So a 3x3 conv costs 6 matmul slots per 512-px chunk instead of 9.
Two images (A, B) run concurrently on disjoint PE-array column halves
(tile_position (0,0) vs (0,64)), so the array stays fully busy.

v2 front-latency restructure (the v1 kernel spent 39us of 68us before the
first conv matmul):
- X2 tiles are packed on the HOST (pre-cast bf16, pre-shifted, pre-padded)
  and DMA'd directly: no on-device fp32->bf16 casts (was 15us of ScalarE),
  no SBUF->SBUF shift DMAs, and half the HBM bytes for x.
- base (expert conv weights in stacked-tap layout) uploads in bf16 (1MB not
  2MB); the combined-weight MACs accumulate in fp32 on VectorE (rows 0-1
  taps) and GpSimdE (row-2 taps) in parallel, reading the routing weights
  straight from PSUM (no broadcast-copy hop).
- The routing chain drops the r/emb transposes (||r|| comes from the diagonal
  of a PE Gram matmul), uses ALU divide instead of reciprocal+mult hops, and
  skips the softmax max-subtraction (cosine sims are bounded by 1).
- The constant blob is split so the 273KB the routing projection needs lands
  first; x and base follow in compute order.
- Output drains PSUM->SBUF as bf16 (host upcasts) halving the store traffic;
  the last chunk is 384 px wide (flat rows 62-63 are VALID-conv garbage).

The flat-pixel formulation computes 64x62 output positions per image; the
garbage columns (VALID conv is 62x62) are trimmed on the host.

Hardware/toolchain constraints honored:
- A Matmult can carry only ONE semaphore wait (walrus ISA): Bacc's
  compile() passes legalize the rest, and PE-queue NOPs with dependency APs
  (the Tile-sanctioned mechanism) absorb cross-engine waits up front.
- HWDGE DMA descriptors only get the fast 16-engine spray for full-tile-width
  destinations with non-overlapping source rows; everything else runs ~10x
  slower, so all loads are shaped accordingly.
- fp32r matmuls are rejected at tile_position != (0,0), hence bf16.
"""

import functools
import os
import sys

import numpy as np

for _p in ("/opt/trn_rl_repo",):
    if os.path.isdir(_p) and _p not in sys.path:
        sys.path.insert(0, _p)

import ml_dtypes

import concourse.bacc as bacc
import concourse.bass as bass
import concourse.mybir as mybir
import concourse.tile as tile
from concourse.bass_utils import run_bass_kernel_spmd

FP = mybir.dt.float32
BF = mybir.dt.bfloat16
AF = mybir.ActivationFunctionType
OP = mybir.AluOpType
BF_NP = ml_dtypes.bfloat16

N_CORES = 8
B = 32
B_LOC = B // N_CORES          # images per core
NPAIR = B_LOC // 2
CIN = 64
COUT = 64
PIX = 64 * 64                 # flat pixels computed per image (incl. garbage)
XCOLS = 4160                  # X2 columns (max read col 4097, zero-padded)
NEXP = 10
D = 128
R = 512
CWF = 384                     # combined weights: 192 pair-taps + 192 row-2
CW_V = 192                    # VectorE's MAC share: rows-0/1 taps, 128 parts
# chunk list: (col offset, width). Last chunk is 384 wide: flat rows 62/63
# (cols 3968:4096) are VALID-conv garbage, never stored.
CHUNKS = [(c * 512, 512) for c in range(7)] + [(3584, 384)]
WAVE = 3                      # chunks per wave (PSUM banks: 6 conv + 2 routing)

# csta columns (fp32): what the routing projection needs, lands first
CA_RPW = 0                    # [128, 4, 128]
CA_RV = 512                   # [128, 4, 4]
CA_RPB = 528                  # [128, 1]
CA_DIAG = 529                 # [4, 4] identity mask (Gram diagonal extract)
CA_COLS = 533
# cstb columns (fp32): the rest of the routing constants
CB_ID = 0                     # [16, 16] identity (10x10 used for transpose)
CB_EMB = 16                   # [10, 128] on partitions 0..9
CB_SELP = 144                 # [4, 2, 128] pair selector, partitions 0..3
CB_SELI = 400                 # [4, 4, 128] image selector, partitions 0..3
CB_CBT = 912                  # [128, 10] conv_b.T tiled 2x
CB_COLS = 922


def build_nc():
    # Bacc (not raw Bass): its compile() runs move_matmul_waits_to_ldweights +
    # generate_event_semaphores, which legalize multi-wait instructions for
    # the walrus ISA (each instruction carries at most one sync wait).
    nc = bacc.Bacc(None)

    x2_d = nc.dram_tensor("x2", [B_LOC * 128, XCOLS], BF, kind="ExternalInput")
    csta_d = nc.dram_tensor("csta", [128, CA_COLS], FP, kind="ExternalInput")
    cstb_d = nc.dram_tensor("cstb", [128, CB_COLS], FP, kind="ExternalInput")
    identb_d = nc.dram_tensor("identb", [128, 128], BF, kind="ExternalInput")
    base_d = nc.dram_tensor("base", [128, NEXP, CWF], BF,
                            kind="ExternalInput")
    out_d = nc.dram_tensor("out", [B_LOC, COUT, PIX], BF, kind="ExternalOutput")

    with tile.TileContext(nc) as tc:
        with (
            tc.tile_pool(name="consts", bufs=1) as consts,
            tc.tile_pool(name="x2p", bufs=4) as x2p,
            tc.tile_pool(name="cwbp", bufs=4) as cwbp,
            tc.tile_pool(name="outp", bufs=2) as outp,
            tc.tile_pool(name="scr", bufs=1) as scr,
            tc.tile_pool(name="rps", bufs=2, space="PSUM") as rps,
            tc.tile_pool(name="cwps", bufs=1, space="PSUM") as cwps,
            tc.tile_pool(name="cps", bufs=5, space="PSUM") as cps,
        ):
            # activation-table warmup: pulls the lazy Sqrt/Exp table loads
            # (1.3 us each) off the routing critical path.  Exp first: the
            # routing path uses Sqrt (twice) before its Exp, so warm in the
            # order that leaves Sqrt resident.  memset on GpSimd: it is idle
            # at startup, while VectorE's queue would delay the warmup.
            warm = scr.tile([1, 1], FP)
            nc.gpsimd.memset(warm, 1.0)
            nc.scalar.activation(out=warm, in_=warm, func=AF.Exp)
            nc.scalar.activation(out=warm, in_=warm, func=AF.Sqrt)

            # ---- input DMAs ---------------------------------------------
            # csta (the routing projection's inputs) goes ALONE first: the
            # DMA engines share ~280GB/s across all in-flight transfers, and
            # letting x2/base contend pushed csta (and thus the whole
            # routing->weights->conv chain) 4us later.  A sync-queue NOP
            # gates the bulk issues on csta's completion.
            csta = consts.tile([128, CA_COLS], FP)
            nc.sync.dma_start(out=csta, in_=csta_d[:])
            cstb = consts.tile([128, CB_COLS], FP)
            identb = consts.tile([128, 128], BF)
            base_t = consts.tile([128, NEXP, CWF], BF)
            x2 = [x2p.tile([128, XCOLS], BF, name="x2", tag="x2")
                  for _ in range(B_LOC)]
            # the gate NOP "writes" one element of every gated destination:
            # writer-writer ordering is what actually binds under the
            # work-conserving scheduler (a bare NOP just gets reordered).
            nc.sync.dma_start(out=cstb, in_=cstb_d[:])
            nc.sync.dma_start(out=identb, in_=identb_d[:])
            dgate = mybir.InstNoOp(
                name=nc.get_next_instruction_name(), text_hint="dgate",
                ins=[nc.sync.lower_ap(csta[:, 0:1])],
                outs=[nc.sync.lower_ap(t[0:1, 0:1])
                      for t in (base_t, *x2)])
            nc.sync.add_instruction(dgate)
            nc.sync.dma_start(out=base_t, in_=base_d[:])
            xfull = x2_d[:]
            for i in range(B_LOC):
                nc.sync.dma_start(out=x2[i], in_=bass.AP(
                    tensor=xfull.tensor,
                    offset=xfull.offset + i * 128 * XCOLS,
                    ap=[[XCOLS, 128], [1, XCOLS]]))

            rpw_t = csta[:, CA_RPW:CA_RPW + 512].rearrange(
                "p (k d) -> p k d", k=4)
            rv_t = csta[:, CA_RV:CA_RV + 16].rearrange("p (k b) -> p k b", k=4)
            rpb_t = csta[:, CA_RPB:CA_RPB + 1]
            diag_t = csta[0:B_LOC, CA_DIAG:CA_DIAG + B_LOC]
            ident = cstb[0:16, CB_ID:CB_ID + 16]
            emb_t = cstb[0:NEXP, CB_EMB:CB_EMB + 128]
            selp_t = cstb[0:B_LOC, CB_SELP:CB_SELP + 256].rearrange(
                "b (p q) -> b p q", p=NPAIR)
            seli_t = cstb[0:B_LOC, CB_SELI:CB_SELI + 512].rearrange(
                "b (i q) -> b i q", i=B_LOC)
            cbt_t = cstb[:, CB_CBT:CB_CBT + NEXP]

            # normalized embeddings FIRST on the Vector queue (they only
            # need cstb, which lands ~2us before the rpw matmuls finish --
            # emitting them after radd cost 3.5us of head-of-line blocking)
            esq = scr.tile([NEXP, D], FP)
            en2 = scr.tile([NEXP, 1], FP)
            nc.vector.scalar_tensor_tensor(out=esq, in0=emb_t, scalar=1.0,
                                           in1=emb_t, op0=OP.mult, op1=OP.mult,
                                           accum_out=en2)
            enorm = scr.tile([NEXP, 1], FP)
            nc.scalar.activation(out=enorm, in_=en2, func=AF.Sqrt)
            einv = scr.tile([NEXP, 1], FP)
            nc.vector.reciprocal(einv, enorm)
            ehat = scr.tile([NEXP, D], FP)
            nc.vector.tensor_scalar(out=ehat, in0=emb_t, scalar1=einv,
                                    scalar2=None, op0=OP.mult)

            # ---- routing: r = rv @ rp_w.T + rp_b  (D on partitions) -------
            r_ps = rps.tile([128, B_LOC], FP, tag="r")
            for k0 in range(R // 128):
                nc.tensor.matmul(r_ps, lhsT=rpw_t[:, k0, :], rhs=rv_t[:, k0, :],
                                 start=(k0 == 0), stop=(k0 == R // 128 - 1))
            # emb transpose next on PE: its input is ready before radd lands
            ehatT_ps = rps.tile([D, NEXP], FP, tag="r")
            nc.tensor.transpose(ehatT_ps, ehat, ident[:NEXP, :NEXP])
            ehatT = scr.tile([D, NEXP], FP)
            nc.vector.tensor_copy(ehatT, ehatT_ps)
            rT = scr.tile([128, B_LOC], FP)
            nc.vector.tensor_scalar(out=rT, in0=r_ps, scalar1=rpb_t,
                                    scalar2=None, op0=OP.add)

            # ||r_b||^2 from the diagonal of the Gram matrix rT.T @ rT
            # (avoids the PE transpose + PSUM copy + square of v1)
            gram_ps = rps.tile([B_LOC, B_LOC], FP, tag="r")
            nc.tensor.matmul(gram_ps, lhsT=rT, rhs=rT, start=True, stop=True)
            gscrap = scr.tile([B_LOC, B_LOC], FP)
            rn2 = scr.tile([B_LOC, 1], FP)
            nc.vector.scalar_tensor_tensor(out=gscrap, in0=gram_ps, scalar=1.0,
                                           in1=diag_t, op0=OP.mult,
                                           op1=OP.mult, accum_out=rn2)
            rnorm = scr.tile([B_LOC, 1], FP)
            nc.scalar.activation(out=rnorm, in_=rn2, func=AF.Sqrt)

            # cosine sim [b, n]: the 1/||r|| scale is FUSED into the Exp
            # activation (per-partition scale, reading the dot straight from
            # PSUM); softmax skips the max-subtraction (|sim| <= 1).
            dot_ps = rps.tile([B_LOC, NEXP], FP, tag="r")
            nc.tensor.matmul(dot_ps, lhsT=rT, rhs=ehatT, start=True, stop=True)
            rinv = scr.tile([B_LOC, 1], FP)
            nc.vector.reciprocal(rinv, rnorm)
            ex = scr.tile([B_LOC, NEXP], FP)
            sume = scr.tile([B_LOC, 1], FP)
            nc.scalar.activation(out=ex, in_=dot_ps, func=AF.Exp,
                                 scale=rinv[:, 0:1], accum_out=sume)
            # softmax normalization is FOLDED into the output drain (psum *
            # sinv_pair + bias) and the bias accumulate: the selector
            # broadcasts consume the raw exp() weights.

            # routing weights broadcast to all 128 partitions via selector
            # matmuls, in consumption order.  TWO psum tiles (img0, img1 |
            # img2, img3, pair0, pair1) so the first pair's combines see
            # their dependency satisfied after two matmuls, not six
            # (sub-tile writer tracking is tile-granular).
            w128a_ps = rps.tile([128, 2, NEXP], FP, tag="r")
            w128b_ps = rps.tile([128, 4, NEXP], FP, tag="r")
            nc.tensor.matmul(w128a_ps[:, 0, :], lhsT=seli_t[:, 0, :],
                             rhs=ex, start=True, stop=True)
            nc.tensor.matmul(w128a_ps[:, 1, :], lhsT=seli_t[:, 1, :],
                             rhs=ex, start=True, stop=True)

            # img0's raw weights staged to SBUF for the ScalarE wI builds --
            # FIRST on the Vector queue: it gates the PE-side img0 combine.
            sinvp = consts.tile([128, NPAIR], FP)
            sump = scr.tile([128, NPAIR], FP)
            w128sb = consts.tile([128, NEXP], FP)
            nc.vector.tensor_copy(w128sb, w128a_ps[:, 0, :])

            # ---- combined conv weights ----------------------------------
            # img0 on the (otherwise idle) PE: 10 PSUM-accumulating matmuls
            # with lhsT = w_n * I (built by ScalarE from the bf16 identity),
            # rhs = base_n.  Runs concurrently with img1's VectorE MAC chain,
            # halving the latency to the first conv matmul.
            wip = scr.tile([128, NEXP, 128], BF)
            cw_ps = cwps.tile([128, CWF], FP, name="cw0")
            for n in range(NEXP):
                nc.scalar.activation(out=wip[:, n, :], in_=identb,
                                     func=AF.Copy, scale=w128sb[:, n:n + 1])
                nc.tensor.matmul(cw_ps, lhsT=wip[:, n, :],
                                 rhs=base_t[:, n, :],
                                 start=(n == 0), stop=(n == NEXP - 1))
            cwb0 = cwbp.tile([128, CWF], BF, name="cwb", tag="cwb")
            nc.scalar.activation(out=cwb0, in_=cw_ps, func=AF.Copy)

            # imgs 2/3 + pair selector matmuls AFTER the wI matmuls: the PE
            # completion semaphore coalesces increments, so img1's Vector
            # chain (gated on sel-a's write) would otherwise not see its
            # dependency satisfied until ALL six selector matmuls ran.
            nc.tensor.matmul(w128b_ps[:, 0, :], lhsT=seli_t[:, 2, :],
                             rhs=ex, start=True, stop=True)
            nc.tensor.matmul(w128b_ps[:, 1, :], lhsT=seli_t[:, 3, :],
                             rhs=ex, start=True, stop=True)
            nc.tensor.matmul(w128b_ps[:, 2, :], lhsT=selp_t[:, 0, :],
                             rhs=ex, start=True, stop=True)
            nc.tensor.matmul(w128b_ps[:, 3, :], lhsT=selp_t[:, 1, :],
                             rhs=ex, start=True, stop=True)

            # imgs 1-3 as all-bf16 MAC chains on VectorE, reading the raw
            # weights straight from PSUM.  NOP gates keep the work-conserving
            # scheduler from interleaving the chains (which would delay
            # img1, the conv gate, by several us).  The pair-stacked softmax
            # denominators (free-axis sum of the pair selector outputs) and
            # the folded-normalization bias accumulates slot in after the
            # img1/img2 chains -- their consumers (the drains) run much
            # later.
            bias2 = consts.tile([128, NPAIR], FP)
            bscrap = scr.tile([128, NEXP], FP)
            bscrap2 = scr.tile([128, NEXP], FP)
            wslot = [(w128a_ps, 1), (w128b_ps, 0), (w128b_ps, 1)]
            cwb = [cwb0]
            for i in (1, 2, 3):
                wtile, slot = wslot[i - 1]
                cwbi = cwbp.tile([128, CWF], BF, name="cwb", tag="cwb")
                if i >= 2:
                    # serialize img2/img3's chains behind img1's (the conv
                    # gate): a 1-element copy from cwb1 into this tile is a
                    # writer-writer ordering the scheduler cannot hoist --
                    # a bare NOP gate gets reordered and the interleaved
                    # chains delayed cwb1 by 3.5us.  img2 and img3 still
                    # interleave with each other, which pipelines their
                    # semaphore latency.
                    nc.vector.tensor_copy(cwbi[0:1, 0:1], cwb[1][0:1, 0:1])
                nc.vector.tensor_scalar(out=cwbi, in0=base_t[:, 0, :],
                                        scalar1=wtile[:, slot, 0:1],
                                        scalar2=None, op0=OP.mult)
                for n in range(1, NEXP):
                    nc.vector.scalar_tensor_tensor(
                        out=cwbi, in0=base_t[:, n, :],
                        scalar=wtile[:, slot, n:n + 1],
                        in1=cwbi, op0=OP.mult, op1=OP.add)
                cwb.append(cwbi)
                if i == 1:
                    nc.vector.tensor_reduce(
                        out=sump[:, 0:1], in_=w128b_ps[:, 2, :],
                        axis=mybir.AxisListType.X, op=OP.add)
                    nc.vector.reciprocal(sinvp[:, 0:1], sump[:, 0:1])
                    nc.vector.scalar_tensor_tensor(
                        out=bscrap, in0=w128b_ps[:, 2, :],
                        scalar=sinvp[:, 0:1],
                        in1=cbt_t, op0=OP.mult, op1=OP.mult,
                        accum_out=bias2[:, 0:1])
                if i == 2:
                    nc.vector.tensor_reduce(
                        out=sump[:, 1:2], in_=w128b_ps[:, 3, :],
                        axis=mybir.AxisListType.X, op=OP.add)
                    nc.vector.reciprocal(sinvp[:, 1:2], sump[:, 1:2])
                    nc.vector.scalar_tensor_tensor(
                        out=bscrap2, in0=w128b_ps[:, 3, :],
                        scalar=sinvp[:, 1:2],
                        in1=cbt_t, op0=OP.mult, op1=OP.mult,
                        accum_out=bias2[:, 1:2])

            # ---- per-pair conv ------------------------------------------
            for p in range(NPAIR):
                iA, iB = 2 * p, 2 * p + 1
                outt = outp.tile([128, PIX], BF)
                dst = out_d[2 * p:2 * p + 2].flatten_outer_dims()
                for w0 in range(0, len(CHUNKS), WAVE):
                    chunks = CHUNKS[w0:w0 + WAVE]
                    pst = {c: cps.tile([128, w], FP, name="pst")
                           for (c, w) in chunks}
                    # PE-queue NOP absorbs all cross-engine waits (psum bank
                    # release, X2 DMAs, cwb MACs) so each Matmult needs at
                    # most its single legal wait.  x2/cwb edges only on the
                    # first wave: later waves are ordered behind it on the
                    # in-order PE queue, and fewer event-semaphore edges
                    # shorten the fixed end-of-NEFF semaphore-reset teardown.
                    ins = []
                    if w0 == 0:
                        ins = [nc.tensor.lower_ap(x2[iA][:, 0:1]),
                               nc.tensor.lower_ap(x2[iB][:, 0:1]),
                               nc.tensor.lower_ap(cwb[iA][:, 0:1]),
                               nc.tensor.lower_ap(cwb[iB][:, 0:1])]
                    dep = mybir.InstNoOp(
                        name=nc.get_next_instruction_name(), text_hint="dep",
                        ins=ins,
                        outs=[nc.tensor.lower_ap(pst[c]) for (c, w) in chunks],
                    )
                    nc.tensor.add_instruction(dep)
                    # phase 1: kernel rows 0+1 in one K=128 pass per dx
                    for dx in range(3):
                        for (c, w) in chunks:
                            lo = c + dx
                            for half, img in ((0, iA), (1, iB)):
                                sl = slice(64 * half, 64 * half + 64)
                                nc.tensor.matmul(
                                    pst[c][sl, :],
                                    lhsT=cwb[img][0:128, dx * 64:dx * 64 + 64],
                                    rhs=x2[img][0:128, lo:lo + w],
                                    start=(dx == 0), stop=False,
                                    skip_group_check=True)
                    # phase 2: kernel row 2, K=64 from the top half only
                    # (weights always on array rows 0-63: tile positions
                    # beyond (0,0)/(0,64) proved unreliable on silicon)
                    for dx in range(3):
                        for (c, w) in chunks:
                            lo = c + 128 + dx
                            for half, img in ((0, iA), (1, iB)):
                                sl = slice(64 * half, 64 * half + 64)
                                nc.tensor.matmul(
                                    pst[c][sl, :],
                                    lhsT=cwb[img][0:64,
                                                  192 + dx * 64:256 + dx * 64],
                                    rhs=x2[img][0:64, lo:lo + w],
                                    start=False, stop=(dx == 2),
                                    skip_group_check=True)
                    for (c, w) in chunks:
                        nc.scalar.activation(
                            out=outt[:, c:c + w],
                            in_=pst[c], func=AF.Identity,
                            bias=bias2[:, p:p + 1],
                            scale=sinvp[:, p:p + 1])
                    lo = chunks[0][0]
                    hi = chunks[-1][0] + chunks[-1][1]
                    nc.sync.dma_start(out=dst[:, lo:hi], in_=outt[:, lo:hi])

    nc.compile()
    return nc


@functools.lru_cache(maxsize=1)
def _nc_cached():
    return build_nc()


def _prep_in_maps(inputs):
    x = np.asarray(inputs["x"], dtype=np.float32).reshape(B, CIN, PIX)
    rv = np.asarray(inputs["routing_vector"], dtype=np.float32)
    conv_w = np.asarray(inputs["conv_w"], dtype=np.float32)
    conv_b = np.asarray(inputs["conv_b"], dtype=np.float32)
    emb = np.asarray(inputs["emb"], dtype=np.float32)
    rp_w = np.asarray(inputs["rp_w"], dtype=np.float32)
    rp_b = np.asarray(inputs["rp_b"], dtype=np.float32)

    # base layout for the stacked-tap lhsT (see module docstring):
    #   cols 0:192  : [p = cin + 64*dy(0/1), n, dx*64 + cout]
    #   cols 192:288: [p = cin (0..63),      n, dx*64 + cout]  (kernel row 2)
    base = np.zeros((128, NEXP, CWF), np.float32)
    b01 = conv_w[:, :, :, 0:2, :].transpose(3, 2, 0, 4, 1)  # dy,c,n,dx,m
    base[:, :, 0:192] = b01.reshape(128, NEXP, 192)
    b2 = conv_w[:, :, :, 2, :].transpose(2, 0, 3, 1)        # c,n,dx,m
    base[0:64, :, 192:384] = b2.reshape(64, NEXP, 192)
    base = base.astype(BF_NP)

    csta = np.zeros((128, CA_COLS), np.float32)
    csta[:, CA_RPW:CA_RPW + 512] = (
        rp_w.T.reshape(4, 128, D).transpose(1, 0, 2).reshape(128, 512))
    csta[:, CA_RPB] = rp_b
    csta[0:B_LOC, CA_DIAG:CA_DIAG + B_LOC] = np.eye(B_LOC, dtype=np.float32)

    cstb = np.zeros((128, CB_COLS), np.float32)
    cstb[0:16, CB_ID:CB_ID + 16] = np.eye(16, dtype=np.float32)
    cstb[0:NEXP, CB_EMB:CB_EMB + 128] = emb
    selp = np.zeros((B_LOC, NPAIR, 128), np.float32)
    for p in range(NPAIR):
        selp[2 * p, p, 0:64] = 1.0
        selp[2 * p + 1, p, 64:128] = 1.0
    cstb[0:B_LOC, CB_SELP:CB_SELP + 256] = selp.reshape(B_LOC, 256)
    seli = np.zeros((B_LOC, B_LOC, 128), np.float32)
    for i in range(B_LOC):
        seli[i, i, :] = 1.0
    cstb[0:B_LOC, CB_SELI:CB_SELI + 512] = seli.reshape(B_LOC, 512)
    cstb[:, CB_CBT:CB_CBT + NEXP] = np.tile(conv_b.T, (2, 1))

    in_maps = []
    for c in range(N_CORES):
        sl = slice(B_LOC * c, B_LOC * (c + 1))
        xr = x[sl]                                     # [B_LOC, 64, PIX]
        x2 = np.zeros((B_LOC, 128, XCOLS), np.float32)
        x2[:, 0:64, 0:PIX] = xr
        x2[:, 64:128, 0:PIX - 64] = xr[:, :, 64:]
        ccsta = csta.copy()
        ccsta[:, CA_RV:CA_RV + 16] = (
            rv[sl].T.reshape(4, 128, B_LOC).transpose(1, 0, 2).reshape(128, 16))
        in_maps.append({
            "x2": x2.astype(BF_NP).reshape(B_LOC * 128, XCOLS),
            "csta": ccsta,
            "cstb": cstb,
            "identb": np.eye(128, dtype=BF_NP),
            "base": base,
        })
    return in_maps


def run(inputs, trace=False, **kw):
    """Returns (full_output, BassKernelResults)."""
    nc = _nc_cached()
    in_maps = _prep_in_maps(inputs)
    res = run_bass_kernel_spmd(nc, in_maps, core_ids=list(range(N_CORES)),
                               trace=trace, **kw)
    outs = [np.asarray(r["out"]).astype(np.float32)
            .reshape(B_LOC, COUT, 64, 64)[:, :, :62, :62]
            for r in res.results]
    return np.concatenate(outs, axis=0), res


def kernel(**inputs):
    out, _ = run(inputs, trace=False)
    return out
